# revision 13
# baseline (speedup 1.0000x reference)
"""Trainium2 Bass kernel for nn_DefaultSegmentLinear.

Computes out[M, N] = (x[M, K] @ W[N, K]^T) * (s_x * s_w[chunk]) + bias[N]
with M=8192, K=4096, N=4096 (C=4 chunks of 1024 out-features).

Strategy
--------
- Mixed precision over the contraction dim: the first KB k-tiles run in
  bf16, the remaining KF k-tiles run in fp8 e4m3 with
  perf_mode=DoubleRow (2 MACs/cell/cycle; the PE packs two fp8 weights
  per cell, contracting 256 k per instruction). Measured rel err of the
  16/16 split is ~1.65e-2 vs the 2e-2 gate (fp8 quantization noise of
  half the k-sum; bf16 alone is 1.5e-3, pure fp8 2.4e-2).
- Weights stay UNIT scale (folding the <1 scales into fp8 weights
  would push values toward e4m3's subnormals); the per-chunk scale
  s_x*s_w[c] and the bias are applied at the PSUM drain by the ACT
  engine (out = psum*scale + bias, both per-partition operands).
- Sharding: M sharded 8 ways. Each core keeps its x^T slice resident
  in SBUF (bf16 part [KB*128, 1024] + fp8 part [KF*128, 1024] ~ 6 MiB
  at 16/16), streams W^T once (~24 MiB), writes out^T fp32 (16 MiB).
- PSUM regions are [128 n-cols, 1024 m] fp32 = 2 banks, so bf16
  matmuls use 1024-wide moving operands (the bf16/fp8 moving cap,
  halving instruction+LDWEIGHTS overhead vs 512) and fp8 DoubleRow
  matmuls share one 256-col weight load across the two 512-m halves.
  4 regions cycle through the 8 PSUM banks; consecutive n-blocks
  alternate region pairs so fresh matmuls never wait on drains.
- Output is produced transposed ([N, M] per core); the host
  concatenates the 8 core slices and transposes back.
"""

import os

import numpy as np
import ml_dtypes

import concourse.bacc as bacc
import concourse.mybir as mybir
import concourse.tile as tile
from concourse import bass_utils

P = 128
M, K, N = 8192, 4096, 4096
N_CORES = 8
MC = M // N_CORES           # 1024 rows of x per core
KT = K // P                 # 32 k-tiles
KB = int(os.environ.get("KERNEL_KB", "16"))  # bf16 k-tiles (low k)
KF = KT - KB                # fp8 k-tiles (high k)
KP = KF // 2                # fp8 DoubleRow pairs
NB = 256                    # n-block width (2 psum regions)
NBLK = N // NB              # 16 n-blocks
NSUB = NB // P              # 2 region subtiles per block

F32 = mybir.dt.float32
BF16 = mybir.dt.bfloat16
FP8 = mybir.dt.float8e4

_CACHE: dict = {}


def _build(iters: int = 1):
    """Build + compile the per-core Bass program.

    iters > 1 wraps the body in a hardware loop (for timing runs).
    """
    nc = bacc.Bacc("TRN2", target_bir_lowering=False, debug=False)
    xbT_d = nc.dram_tensor("xbT", [KB * P, MC], BF16, kind="ExternalInput").ap() \
        if KB else None
    xfT_d = nc.dram_tensor("xfT", [KF * P, MC], FP8, kind="ExternalInput").ap() \
        if KF else None
    # W pre-arranged host-side into per-block SBUF images so every W DMA
    # is a fully contiguous [P, nKB] copy. The fp8 image additionally
    # carries the DoubleRowSwInterleave layout (A/B k-slab pairs
    # interleaved per column, columns reversed) so the weight load is a
    # single contiguous 256-element read (FWL) instead of DoubleRow's
    # two reversed 128-column passes.
    wbT_d = nc.dram_tensor("wbA", [NBLK, P, KB * NB], BF16,
                           kind="ExternalInput").ap() if KB else None
    wfT_d = nc.dram_tensor("wfA", [NBLK, P, KP * NSUB * 2 * P], FP8,
                           kind="ExternalInput").ap() if KF else None
    # bias/scale pre-arranged host-side as [128, N/128]: column j holds
    # bias[j*128:(j+1)*128] / scale for chunk(j) (per-partition scalars
    # for the ACT drain).
    bias_d = nc.dram_tensor("biasc", [P, N // P], F32, kind="ExternalInput").ap()
    scale_d = nc.dram_tensor("scalec", [P, N // P], F32, kind="ExternalInput").ap()
    outT_d = nc.dram_tensor("outT", [N, MC], F32, kind="ExternalOutput").ap()

    with tile.TileContext(nc) as tc:
        with (
            tc.tile_pool(name="xres", bufs=max(KB, 1) + max(KP, 1)) as xres_pool,
            tc.tile_pool(name="wbstream", bufs=3) as wb_pool,
            tc.tile_pool(name="wfstream", bufs=3) as wf_pool,
            tc.tile_pool(name="biasp", bufs=2) as bias_pool,
            tc.tile_pool(name="ostage", bufs=4) as o_pool,
            tc.tile_pool(name="psum", bufs=4, space="PSUM") as psum_pool,
        ):
            def body(it):
                bias_sb = bias_pool.tile([P, N // P], F32, name=f"biassb{it}")
                nc.sync.dma_start(bias_sb[:], bias_d[:])
                scale_sb = bias_pool.tile([P, N // P], F32, name=f"scalesb{it}")
                nc.sync.dma_start(scale_sb[:], scale_d[:])
                xb_res = [xres_pool.tile([P, MC], BF16, tag="xb",
                                         name=f"xb{it}_{kt}")
                          for kt in range(KB)]
                xf_res = [xres_pool.tile([P, 2, MC], FP8, tag="xf",
                                         name=f"xf{it}_{kp}")
                          for kp in range(KP)]

                def xb_load(kts):
                    for kt in kts:
                        nc.sync.dma_start(
                            xb_res[kt][:], xbT_d[kt * P:(kt + 1) * P, :])

                def xf_load(kps):
                    for kp in kps:
                        nc.sync.dma_start(
                            xf_res[kp][:],
                            xfT_d[kp * 2 * P:(kp + 1) * 2 * P, :].rearrange(
                                "(two p) m -> p two m", p=P))

                w_tiles = {}

                def w_load(nb):
                    if KB:
                        w_b = wb_pool.tile([P, KB, NB], BF16, tag="wb",
                                           name=f"wb{it}_{nb}")
                        nc.sync.dma_start(
                            w_b[:],
                            wbT_d[nb].rearrange("p (kb n) -> p kb n", n=NB))
                    else:
                        w_b = None
                    if KP:
                        w_f = wf_pool.tile([P, KP, NSUB, 2, P], FP8, tag="wf",
                                           name=f"wf{it}_{nb}")
                        nc.sync.dma_start(
                            w_f[:],
                            wfT_d[nb].rearrange(
                                "p (kp nt two n) -> p kp nt two n",
                                kp=KP, nt=NSUB, two=2))
                    else:
                        w_f = None
                    w_tiles[nb] = (w_b, w_f)

                # Interleave the first W block into the x reloads so a
                # fresh pass starts its W stream early.
                xb_load(range(0, min(4, KB)))
                w_load(0)
                xb_load(range(4, KB))
                xf_load(range(KP))
                w_load(1)

                for nb in range(NBLK):
                    if nb + 2 < NBLK:
                        w_load(nb + 2)
                    w_b, w_f = w_tiles.pop(nb)
                    psums = [
                        psum_pool.tile([P, MC], F32, tag="ps",
                                       name=f"ps{it}_{nb}_{nt}")
                        for nt in range(NSUB)
                    ]
                    for kt in range(KB):
                        for nt in range(NSUB):
                            for mh in range(2):
                                nc.tensor.matmul(
                                    psums[nt][:, mh * 512:(mh + 1) * 512],
                                    w_b[:, kt, nt * P:(nt + 1) * P],
                                    xb_res[kt][:, mh * 512:(mh + 1) * 512],
                                    start=(kt == 0),
                                    stop=(kt == KB - 1 and KP == 0),
                                    skip_group_check=True,
                                )
                    for kp in range(KP):
                        for nt in range(NSUB):
                            for mh in range(2):
                                nc.tensor.matmul(
                                    psums[nt][:, mh * 512:(mh + 1) * 512],
                                    w_f[:, kp, nt, :, :],
                                    xf_res[kp][:, :, mh * 512:(mh + 1) * 512],
                                    start=(KB == 0 and kp == 0),
                                    stop=(kp == KP - 1),
                                    perf_mode=(
                                        mybir.MatmulPerfMode
                                        .DoubleRowSwInterleave),
                                    skip_group_check=True,
                                )
                    for nt in range(NSUB):
                        ncol = nb * NSUB + nt
                        o_sb = o_pool.tile([P, MC], F32, tag="o",
                                           name=f"o{it}_{nb}_{nt}")
                        nc.scalar.activation(
                            out=o_sb[:], in_=psums[nt][:],
                            func=mybir.ActivationFunctionType.Identity,
                            bias=bias_sb[:, ncol:ncol + 1],
                            scale=scale_sb[:, ncol:ncol + 1],
                        )
                        nc.scalar.dma_start(
                            outT_d[ncol * P:(ncol + 1) * P, :], o_sb[:])

            if iters == 1:
                body(0)
            else:
                # For_i runs an InstAllEngineBarrier in its per-iteration
                # semaphore-reset block; unroll 4 bodies per iteration and
                # use staggered per-stage resets (one stage per body) so
                # engines never globally drain between timing iterations.
                if iters % 4 == 0:
                    with tc.For_i(0, iters // 4, 1, staggered_reset=True):
                        for u in range(4):
                            if u:
                                tc.stage_boundary()
                            body(u)
                else:
                    with tc.For_i(0, iters, 1):
                        body(0)
    nc.compile()
    return nc


def _prep_inputs(x, w_chunks, bias, input_scale, weight_scales):
    s = (np.float32(input_scale[0]) * weight_scales.astype(np.float32))
    scol = np.repeat(s, N // s.shape[0]).astype(np.float32)   # [N]
    W = w_chunks.reshape(N, K).astype(np.float32)             # unit scale
    WT = np.ascontiguousarray(W.T)                            # [K, N]
    xT = np.ascontiguousarray(x.astype(np.float32).T)         # [K, M]
    kb = KB * P
    xbT = xT[:kb].astype(ml_dtypes.bfloat16)
    xfT = xT[kb:].astype(ml_dtypes.float8_e4m3)
    if KB:
        # per-block SBUF image: wbA[nb, p, kt, nc] = W^T[kt*128+p, nb*256+nc]
        wb = WT[:kb].astype(ml_dtypes.bfloat16)
        wb = wb.reshape(KB, P, NBLK, NB).transpose(2, 1, 0, 3)
        wbA = np.ascontiguousarray(wb).reshape(NBLK, P, KB * NB)
    if KF:
        # DoubleRowSwInterleave image: for each (kp, nt) stationary the
        # 256 contiguous bytes hold j=2*(127-n)+i -> W^T[.., i*128+p, n]
        wf = WT[kb:].astype(ml_dtypes.float8_e4m3)
        wf = wf.reshape(KP, 2, P, NBLK, NSUB, P)   # [kp, i, p, nb, nt, n]
        wf = wf[:, :, :, :, :, ::-1]               # reverse n
        wf = wf.transpose(3, 2, 0, 4, 5, 1)        # [nb, p, kp, nt, n', i]
        wfA = np.ascontiguousarray(wf).reshape(NBLK, P, KP * NSUB * 2 * P)
    bias_c = np.ascontiguousarray(
        bias.astype(np.float32).reshape(N // P, P).T)         # [128, N/128]
    scale_c = np.ascontiguousarray(
        scol.reshape(N // P, P).T)                            # [128, N/128]
    in_maps = []
    for c in range(N_CORES):
        m = {"biasc": bias_c, "scalec": scale_c}
        if KB:
            m["xbT"] = np.ascontiguousarray(xbT[:, c * MC:(c + 1) * MC])
            m["wbA"] = wbA
        if KF:
            m["xfT"] = np.ascontiguousarray(xfT[:, c * MC:(c + 1) * MC])
            m["wfA"] = wfA
        in_maps.append(m)
    return in_maps


def kernel(x, w_chunks, bias, input_scale, weight_scales):
    x = np.asarray(x)
    w_chunks = np.asarray(w_chunks)
    bias = np.asarray(bias)
    input_scale = np.asarray(input_scale)
    weight_scales = np.asarray(weight_scales)
    if "nc" not in _CACHE:
        _CACHE["nc"] = _build(iters=1)
    nc = _CACHE["nc"]
    in_maps = _prep_inputs(x, w_chunks, bias, input_scale, weight_scales)
    res = bass_utils.run_bass_kernel_spmd(
        nc, in_maps, core_ids=list(range(N_CORES)))
    outT = np.concatenate(
        [res.results[c]["outT"] for c in range(N_CORES)], axis=1)  # [N, M]
    return np.ascontiguousarray(outT.T)


# revision 22
# speedup vs baseline: 1.0681x; 1.0681x over previous
"""Trainium2 Bass kernel for nn_DefaultSegmentLinear.

Computes out[M, N] = (x[M, K] @ W[N, K]^T) * (s_x * s_w[chunk]) + bias[N]
with M=8192, K=4096, N=4096 (C=4 chunks of 1024 out-features).

Strategy
--------
- Mixed precision over the contraction dim: the first KB=14 k-tiles
  run in bf16, the remaining KF=18 k-tiles run in fp8 e4m3 with
  perf_mode=DoubleRow (the PE packs two fp8 weights per cell and
  contracts 256 k per instruction; measured 1.95x bf16 per k on HW).
  Measured rel err of the 14/18 split is 1.73e-2 vs the 2e-2 gate
  (fp8 quantization noise of 18/32 of the k-sum; bf16 alone is
  1.5e-3, pure fp8 2.46e-2 — the split is the accuracy-speed dial).
  Calibration (this HW): pure bf16 551 us, 14/18 mixed 402 us, pure
  fp8 282 us per pass; a moving-width-256 diagnostic showed per-MM
  overhead is ~4 cycles, i.e. the kernel sits at the PE stream-rate
  floor and only the bf16:fp8 ratio matters.
- Weights stay UNIT scale (folding the <1 scales into fp8 weights
  would push values toward e4m3's subnormals); the per-chunk scale
  s_x*s_w[c] and the bias are applied at the PSUM drain by the ACT
  engine (out = psum*scale + bias, both per-partition operands).
- Sharding: M sharded 8 ways. Each core keeps its x^T slice resident
  in SBUF (bf16 part [KB*128, 1024] + fp8 part [KF*128, 1024] ~ 6 MiB
  at 16/16), streams W^T once (~24 MiB), writes out^T fp32 (16 MiB).
- PSUM regions are [128 n-cols, 1024 m] fp32 = 2 banks; matmuls write
  512-m halves (the ISA out cap) and each region drains with a single
  ACT instruction. 4 regions cycle through the 8 PSUM banks;
  consecutive n-blocks alternate region pairs so fresh matmuls never
  wait on drains. W arrives as per-block SBUF images prepared on the
  host, so every W DMA is a fully contiguous [128, blockbytes] copy.
- Output is produced transposed ([N, M] per core); the host
  concatenates the 8 core slices and transposes back.
"""

import os

import numpy as np
import ml_dtypes

import concourse.bacc as bacc
import concourse.mybir as mybir
import concourse.tile as tile
from concourse import bass_utils

P = 128
M, K, N = 8192, 4096, 4096
N_CORES = 8
MC = M // N_CORES           # 1024 rows of x per core
KT = K // P                 # 32 k-tiles
KB = int(os.environ.get("KERNEL_KB", "14"))  # bf16 k-tiles (low k)
MSPLIT = int(os.environ.get("KERNEL_MSPLIT", "2"))  # bf16 m-chunks per region
KF = KT - KB                # fp8 k-tiles (high k)
KP = KF // 2                # fp8 DoubleRow pairs
NB = 256                    # n-block width (2 psum regions)
NBLK = N // NB              # 16 n-blocks
NSUB = NB // P              # 2 region subtiles per block

F32 = mybir.dt.float32
BF16 = mybir.dt.bfloat16
FP8 = mybir.dt.float8e4

_CACHE: dict = {}


def _build(iters: int = 1):
    """Build + compile the per-core Bass program.

    iters > 1 wraps the body in a hardware loop (for timing runs).
    """
    nc = bacc.Bacc("TRN2", target_bir_lowering=False, debug=False)
    xbT_d = nc.dram_tensor("xbT", [KB * P, MC], BF16, kind="ExternalInput").ap() \
        if KB else None
    xfT_d = nc.dram_tensor("xfT", [KF * P, MC], FP8, kind="ExternalInput").ap() \
        if KF else None
    # W pre-arranged host-side into per-block SBUF images so every W DMA
    # is a fully contiguous [P, nKB] copy. The fp8 image additionally
    # carries the DoubleRowSwInterleave layout (A/B k-slab pairs
    # interleaved per column, columns reversed) so the weight load is a
    # single contiguous 256-element read (FWL) instead of DoubleRow's
    # two reversed 128-column passes.
    wbT_d = nc.dram_tensor("wbA", [NBLK, P, KB * NB], BF16,
                           kind="ExternalInput").ap() if KB else None
    wfT_d = nc.dram_tensor("wfA", [NBLK, P, KP * NSUB * 2 * P], FP8,
                           kind="ExternalInput").ap() if KF else None
    # bias/scale pre-arranged host-side as [128, N/128]: column j holds
    # bias[j*128:(j+1)*128] / scale for chunk(j) (per-partition scalars
    # for the ACT drain).
    bias_d = nc.dram_tensor("biasc", [P, N // P], F32, kind="ExternalInput").ap()
    scale_d = nc.dram_tensor("scalec", [P, N // P], F32, kind="ExternalInput").ap()
    outT_d = nc.dram_tensor("outT", [N, MC], F32, kind="ExternalOutput").ap()

    with tile.TileContext(nc) as tc:
        with (
            tc.tile_pool(name="xres", bufs=max(KB, 1) + max(KP, 1)) as xres_pool,
            tc.tile_pool(name="wbstream", bufs=3) as wb_pool,
            tc.tile_pool(name="wfstream", bufs=3) as wf_pool,
            tc.tile_pool(name="biasp", bufs=2) as bias_pool,
            tc.tile_pool(name="ostage", bufs=4) as o_pool,
            tc.tile_pool(name="psum", bufs=4, space="PSUM") as psum_pool,
        ):
            def body(it):
                bias_sb = bias_pool.tile([P, N // P], F32, name=f"biassb{it}")
                nc.sync.dma_start(bias_sb[:], bias_d[:])
                scale_sb = bias_pool.tile([P, N // P], F32, name=f"scalesb{it}")
                nc.sync.dma_start(scale_sb[:], scale_d[:])
                xb_res = [xres_pool.tile([P, MC], BF16, tag="xb",
                                         name=f"xb{it}_{kt}")
                          for kt in range(KB)]
                xf_res = [xres_pool.tile([P, 2, MC], FP8, tag="xf",
                                         name=f"xf{it}_{kp}")
                          for kp in range(KP)]

                def xb_load(kts):
                    for kt in kts:
                        nc.sync.dma_start(
                            xb_res[kt][:], xbT_d[kt * P:(kt + 1) * P, :])

                def xf_load(kps):
                    for kp in kps:
                        nc.sync.dma_start(
                            xf_res[kp][:],
                            xfT_d[kp * 2 * P:(kp + 1) * 2 * P, :].rearrange(
                                "(two p) m -> p two m", p=P))

                w_tiles = {}

                def w_load(nb):
                    if KB:
                        w_b = wb_pool.tile([P, KB, NB], BF16, tag="wb",
                                           name=f"wb{it}_{nb}")
                        nc.sync.dma_start(
                            w_b[:],
                            wbT_d[nb].rearrange("p (kb n) -> p kb n", n=NB))
                    else:
                        w_b = None
                    if KP:
                        w_f = wf_pool.tile([P, KP, NSUB, 2, P], FP8, tag="wf",
                                           name=f"wf{it}_{nb}")
                        nc.sync.dma_start(
                            w_f[:],
                            wfT_d[nb].rearrange(
                                "p (kp nt two n) -> p kp nt two n",
                                kp=max(KP, 1), nt=NSUB, two=2))
                    else:
                        w_f = None
                    w_tiles[nb] = (w_b, w_f)

                # Interleave the first W block into the x reloads so a
                # fresh pass starts its W stream early.
                xb_load(range(0, min(4, KB)))
                w_load(0)
                xb_load(range(4, KB))
                xf_load(range(KP))
                w_load(1)

                for nb in range(NBLK):
                    if nb + 2 < NBLK:
                        w_load(nb + 2)
                    w_b, w_f = w_tiles.pop(nb)
                    psums = [
                        psum_pool.tile([P, MC], F32, tag="ps",
                                       name=f"ps{it}_{nb}_{nt}")
                        for nt in range(NSUB)
                    ]
                    mw = MC // MSPLIT
                    for kt in range(KB):
                        for nt in range(NSUB):
                            for mh in range(MSPLIT):
                                nc.tensor.matmul(
                                    psums[nt][:, mh * mw:(mh + 1) * mw],
                                    w_b[:, kt, nt * P:(nt + 1) * P],
                                    xb_res[kt][:, mh * mw:(mh + 1) * mw],
                                    start=(kt == 0),
                                    stop=(kt == KB - 1 and KP == 0),
                                    skip_group_check=True,
                                )
                    for kp in range(KP):
                        for nt in range(NSUB):
                            for mh in range(2):
                                nc.tensor.matmul(
                                    psums[nt][:, mh * 512:(mh + 1) * 512],
                                    w_f[:, kp, nt, :, :],
                                    xf_res[kp][:, :, mh * 512:(mh + 1) * 512],
                                    start=(KB == 0 and kp == 0),
                                    stop=(kp == KP - 1),
                                    perf_mode=mybir.MatmulPerfMode.DoubleRow,
                                    skip_group_check=True,
                                )
                    for nt in range(NSUB):
                        ncol = nb * NSUB + nt
                        o_sb = o_pool.tile([P, MC], F32, tag="o",
                                           name=f"o{it}_{nb}_{nt}")
                        nc.scalar.activation(
                            out=o_sb[:], in_=psums[nt][:],
                            func=mybir.ActivationFunctionType.Identity,
                            bias=bias_sb[:, ncol:ncol + 1],
                            scale=scale_sb[:, ncol:ncol + 1],
                        )
                        nc.scalar.dma_start(
                            outT_d[ncol * P:(ncol + 1) * P, :], o_sb[:])

            if iters == 1:
                body(0)
            else:
                # For_i runs an InstAllEngineBarrier in its per-iteration
                # semaphore-reset block; unroll 4 bodies per iteration and
                # use staggered per-stage resets (one stage per body) so
                # engines never globally drain between timing iterations.
                if iters % 4 == 0:
                    with tc.For_i(0, iters // 4, 1, staggered_reset=True):
                        for u in range(4):
                            if u:
                                tc.stage_boundary()
                            body(u)
                else:
                    with tc.For_i(0, iters, 1):
                        body(0)
    nc.compile()
    return nc


def _prep_inputs(x, w_chunks, bias, input_scale, weight_scales):
    s = (np.float32(input_scale[0]) * weight_scales.astype(np.float32))
    scol = np.repeat(s, N // s.shape[0]).astype(np.float32)   # [N]
    W = w_chunks.reshape(N, K).astype(np.float32)             # unit scale
    WT = np.ascontiguousarray(W.T)                            # [K, N]
    xT = np.ascontiguousarray(x.astype(np.float32).T)         # [K, M]
    kb = KB * P
    xbT = xT[:kb].astype(ml_dtypes.bfloat16)
    xfT = xT[kb:].astype(ml_dtypes.float8_e4m3)
    if KB:
        # per-block SBUF image: wbA[nb, p, kt, nc] = W^T[kt*128+p, nb*256+nc]
        wb = WT[:kb].astype(ml_dtypes.bfloat16)
        wb = wb.reshape(KB, P, NBLK, NB).transpose(2, 1, 0, 3)
        wbA = np.ascontiguousarray(wb).reshape(NBLK, P, KB * NB)
    if KF:
        # DoubleRow image: wfA[nb, p, kp, nt, i, n] = W^T[kb+kp*256+i*128+p,
        # nb*256+nt*128+n] — per-block SBUF image, contiguous DMA.
        wf = WT[kb:].astype(ml_dtypes.float8_e4m3)
        wf = wf.reshape(KP, 2, P, NBLK, NSUB, P)   # [kp, i, p, nb, nt, n]
        wf = wf.transpose(3, 2, 0, 4, 1, 5)        # [nb, p, kp, nt, i, n]
        wfA = np.ascontiguousarray(wf).reshape(NBLK, P, KP * NSUB * 2 * P)
    bias_c = np.ascontiguousarray(
        bias.astype(np.float32).reshape(N // P, P).T)         # [128, N/128]
    scale_c = np.ascontiguousarray(
        scol.reshape(N // P, P).T)                            # [128, N/128]
    in_maps = []
    for c in range(N_CORES):
        m = {"biasc": bias_c, "scalec": scale_c}
        if KB:
            m["xbT"] = np.ascontiguousarray(xbT[:, c * MC:(c + 1) * MC])
            m["wbA"] = wbA
        if KF:
            m["xfT"] = np.ascontiguousarray(xfT[:, c * MC:(c + 1) * MC])
            m["wfA"] = wfA
        in_maps.append(m)
    return in_maps


def kernel(x, w_chunks, bias, input_scale, weight_scales):
    x = np.asarray(x)
    w_chunks = np.asarray(w_chunks)
    bias = np.asarray(bias)
    input_scale = np.asarray(input_scale)
    weight_scales = np.asarray(weight_scales)
    if "nc" not in _CACHE:
        _CACHE["nc"] = _build(iters=1)
    nc = _CACHE["nc"]
    in_maps = _prep_inputs(x, w_chunks, bias, input_scale, weight_scales)
    res = bass_utils.run_bass_kernel_spmd(
        nc, in_maps, core_ids=list(range(N_CORES)))
    outT = np.concatenate(
        [res.results[c]["outT"] for c in range(N_CORES)], axis=1)  # [N, M]
    return np.ascontiguousarray(outT.T)
